# revision 46
# baseline (speedup 1.0000x reference)
"""Trainium2 Bass kernel for nn_ContrastiveLoss (ragged_sequence).

Math (see reference): a cross-attention t2i score matrix scores[i, c] over
B=64 images x B=64 captions, then a max-violation margin loss.

Sharding: captions are sharded 8-per-core across 8 NeuronCores; images are
replicated.  Each core computes its [64, 8] column block of the score
matrix; the tiny 64x64 margin-loss reduction runs on host.

Ragged packing: captions are sorted by length and dealt round-robin to
(core, slot) so slot j holds the 8 captions of global rank 8j..8j+7.  Slots
are padded to two uniform group widths (slots 0-3 -> w0, slots 4-7 -> w1),
so all per-caption word reductions are 2 static strided views instead of 8
ragged ones.  NCW = 4*w0 + 4*w1 (~296 for the reference inputs vs 400
dense) and is kept >= 256 so fp32r matmuls stream at 1 row/cycle.

Device layout (per core):
  Images are processed in 22 "packs" of 3 images (64 real + 2 zero-pad).
  A pack occupies 108 SBUF partitions = 3 images x 36 regions.  The main
  matmul A = im @ s^T is computed with stationary = im-pack [128d, 108br]
  (fp32r, 8 K-chunks of d) and moving = s^T [128d, NCW], giving
  A [108 (b,r), NCW (c,w)] in PSUM.  Word-axis (w) reductions are free-axis
  DVE reduces over the two group views; region-axis (r) reductions are PE
  matmuls:
    - H = Gbd @ E with Gbd = blockdiag(G[b0],G[b1],G[b2]) (Gram matrices)
    - NZ/WZ = ones3^T @ {E*A, E*H} writing pack p's 3 rows of the [66, NCW]
      accumulators directly (ones3 is one constant [108, 3] stationary).
  The softmax normalizer Z cancels in sim = (NZ/Z)/(cn*sqrt(WZ)/Z), so it
  is never computed.  A-matmuls are software-pipelined LOOKAHEAD packs
  ahead so PE's in-order stream never stalls on the ACT/DVE/Pool chain.
  The final log()/lambda and margin loss run on host.
"""

import sys

if "/opt/trn_rl_repo" not in sys.path:
    sys.path.insert(0, "/opt/trn_rl_repo")

import numpy as np

B, R, W, D = 64, 36, 50, 1024
NCORES = 8
CPC = B // NCORES          # captions per core = 8
PACK = 3                   # images per pack
NPACK = 22                 # ceil(64 / 3) -> 66 rows incl. 2 pad images
BP = NPACK * PACK          # 66
PPART = PACK * R           # 108 partitions per pack
KCH = D // 128             # 8 contraction chunks
NGRP = 2                   # caption slot groups (4 slots each)
SPG = CPC // NGRP          # slots per group = 4
MIN_NCW = 256              # fp32r matmul needs moving free size >= 256

MARGIN = 0.2
LAM_SM = 9.0
LAM_LSE = 6.0
EPS = 1e-8

_PROGRAM_CACHE: dict = {}


def _pin_act_tables():
    """Pin activation-function table selection to natural_log_exp_and_others
    (contains Prelu/Square/Ln/Exp/Copy) so the table-load insertion pass never
    ping-pongs between the exp-only and ln-only sets.  Returns a restore fn."""
    import concourse.bacc as bacc
    import concourse.hw_specs as hw_specs

    orig_hw, orig_bacc = hw_specs.get_activation_tables, bacc.get_activation_tables

    def pinned(arch):
        tabs = dict(orig_hw(arch))
        return {
            k: (v if k == "natural_log_exp_and_others" else frozenset())
            for k, v in tabs.items()
        }

    hw_specs.get_activation_tables = pinned
    bacc.get_activation_tables = pinned

    def restore():
        hw_specs.get_activation_tables = orig_hw
        bacc.get_activation_tables = orig_bacc

    return restore


def build_program(w0: int, w1: int, debug: bool = False):
    """Build the per-core Bass/Tile program (identical on all 8 cores).
    w0/w1: padded word capacity of caption slots 0-3 / 4-7."""
    import concourse.bacc as bacc
    import concourse.mybir as mybir
    import concourse.tile as tile

    f32 = mybir.dt.float32
    f32r = mybir.dt.float32r
    AF = mybir.ActivationFunctionType
    ALU = mybir.AluOpType
    AX = mybir.AxisListType

    G0 = SPG * w0              # columns of group 0
    NCW = G0 + SPG * w1
    assert NCW >= MIN_NCW

    restore_tables = _pin_act_tables()
    nc = bacc.Bacc("TRN2", target_bir_lowering=False, debug=debug)

    # const AP for the Ln(s2 + 1e-30) bias (only 0.0/1.0 are pre-registered)
    _c30 = nc.alloc_sbuf_tensor("const-float32-1e-30", [128, 1], f32)
    nc.gpsimd.memset(_c30.ap(), 1e-30)
    nc.const_aps.aps[(f32, 1e-30)] = _c30.ap()
    nc.all_engine_barrier()

    im_d = nc.dram_tensor("im_packed", [NPACK, 128, KCH * PPART], f32, kind="ExternalInput")
    s_d = nc.dram_tensor("s_packed", [128, KCH * NCW], f32, kind="ExternalInput")
    g_d = nc.dram_tensor("gbd", [PPART, NPACK * PPART], f32, kind="ExternalInput")
    o_d = nc.dram_tensor("ones_w", [PPART, 2 * BP], f32, kind="ExternalInput")
    cn_d = nc.dram_tensor("cn66", [BP, NCW], f32, kind="ExternalInput")
    out_d = nc.dram_tensor("rowz8", [B, CPC], f32, kind="ExternalOutput")

    # Four-level software pipeline.  Iteration p issues:
    #   A-matmul(p+1) | Prelu(p) | rnrm(p-1) | an+E(p-2) | ea/H/eh/NZ/WZ(p-3)
    # so every op only consumes results from previous iterations and each
    # engine's in-order stream never waits on the current pack's chain.
    # PSUM: psA holds a_ps(p-3..p+1) = 5 banks, psH 1 (h is consumed by eh
    # within the same stage), accumulators 2 -> 8 banks total.
    LOOKAHEAD = 1
    EPI_DELAY = 3

    def grp(ap, g):
        """[PPART, NCW] access -> group-g 3D view [PPART, SPG, wg]."""
        sl = ap[:, 0:G0] if g == 0 else ap[:, G0:NCW]
        return sl.rearrange("p (c w) -> p c w", c=SPG)

    with tile.TileContext(nc) as tc:
        with (
            tc.tile_pool(name="const", bufs=1) as cpool,
            tc.tile_pool(name="imp", bufs=6) as impool,
            tc.tile_pool(name="work", bufs=8) as work,
            tc.tile_pool(name="small", bufs=8) as small,
            tc.tile_pool(name="ph2", bufs=1) as ph2,
            tc.tile_pool(name="psA", bufs=LOOKAHEAD + EPI_DELAY + 1, space="PSUM") as psA,
            tc.tile_pool(name="psH", bufs=1, space="PSUM") as psH,
            tc.tile_pool(name="psacc", bufs=1, space="PSUM") as psacc,
        ):
            s_sb = cpool.tile([128, KCH * NCW], f32)
            g_all = cpool.tile([PPART, NPACK * PPART], f32)
            o_all = cpool.tile([PPART, 2 * BP], f32)
            cn_sb = cpool.tile([BP, NCW], f32)

            def load_s():
                # 3 chunk DMAs so pack 0's first A-matmuls unblock early
                for c0, c1 in ((0, 3), (3, 6), (6, KCH)):
                    nc.sync.dma_start(
                        s_sb[:, c0 * NCW:c1 * NCW].bitcast(f32r),
                        s_d[:, c0 * NCW:c1 * NCW].bitcast(f32r),
                    )

            def load_go():
                # all Gram blocks in one wide tile (one DMA, full-rate rows);
                # pack p's stationary is a column slice; first use is
                # stage_b(0), EPI_DELAY iterations in.  ones: sliding window,
                # 3 block-ones columns at 63..65 so the [*, 63-3p : 129-3p]
                # slice has them at local columns 3p..3p+2.
                nc.sync.dma_start(g_all[:].bitcast(f32r), g_d[:].bitcast(f32r))
                nc.sync.dma_start(o_all[:].bitcast(f32r), o_d[:].bitcast(f32r))

            # persistent PSUM accumulators for the r-reductions
            nz_acc = psacc.tile([BP, NCW], f32)
            wz_acc = psacc.tile([BP, NCW], f32)

            im_tiles: dict = {}
            a_tiles: dict = {}

            def fetch(p):
                im_sb = impool.tile([128, KCH * PPART], f32, tag="im")
                nc.sync.dma_start(im_sb[:].bitcast(f32r), im_d[p].bitcast(f32r))
                im_tiles[p] = im_sb

            def a_matmul(p):
                # A[108, NCW] = sum_k im_pack_k^T @ s_k  (fp32r)
                im_sb = im_tiles.pop(p)
                a_ps = psA.tile([PPART, NCW], f32)
                for k in range(KCH):
                    nc.tensor.matmul(
                        a_ps[:],
                        im_sb[:, k * PPART:(k + 1) * PPART].bitcast(f32r),
                        s_sb[:, k * NCW:(k + 1) * NCW].bitcast(f32r),
                        start=(k == 0),
                        stop=(k == KCH - 1),
                    )
                a_tiles[p] = a_ps

            e_tiles: dict = {}
            al_tiles: dict = {}
            s2_tiles: dict = {}
            rn_tiles: dict = {}

            def leaky(p):
                """Al(p) = leaky_relu(A, 0.1) on ACT (Prelu)."""
                al = work.tile([PPART, NCW], f32, tag="al")
                nc.scalar.activation(al[:], a_tiles[p][:], AF.Prelu, alpha=0.1)
                al_tiles[p] = al

            def sq_s2(p):
                """s2(p)[108, 8] = sum_w Al^2 (square + group reduces on DVE;
                keeping sq off Pool breaks the sq->...->an->sq stream cycle)."""
                al = al_tiles[p]
                sq = work.tile([PPART, NCW], f32, tag="sq")
                nc.gpsimd.tensor_mul(sq[:], al[:], al[:])
                s2 = small.tile([PPART, CPC], f32, tag="s2")
                nc.vector.tensor_reduce(s2[:, 0:SPG], grp(sq, 0), AX.X, ALU.add)
                nc.vector.tensor_reduce(s2[:, SPG:CPC], grp(sq, 1), AX.X, ALU.add)
                s2_tiles[p] = s2

            def rnrm_stage(p):
                """rnrm(p) = rsqrt(s2 + 1e-30) = exp(-0.5*ln(s2 + 1e-30));
                matches the reference 1/(sqrt(s2)+1e-8) to ~1e-10 rel."""
                s2 = s2_tiles.pop(p)
                lns = small.tile([PPART, CPC], f32, tag="lns")
                nc.scalar.activation(lns[:], s2[:], AF.Ln, bias=1e-30)
                rnrm = small.tile([PPART, CPC], f32, tag="rnrm")
                nc.scalar.activation(rnrm[:], lns[:], AF.Exp, scale=-0.5)
                rn_tiles[p] = rnrm

            def an_e(p):
                """An(p) = Al * rnrm (group-broadcast muls on Pool);
                E(p) = exp(9*An) on ACT."""
                al = al_tiles.pop(p)
                rnrm = rn_tiles.pop(p)
                an = work.tile([PPART, NCW], f32, tag="an")
                nc.vector.tensor_mul(
                    grp(an, 0), grp(al, 0),
                    rnrm[:, 0:SPG].broadcast_to([PPART, SPG, w0]),
                )
                nc.vector.tensor_mul(
                    grp(an, 1), grp(al, 1),
                    rnrm[:, SPG:CPC].broadcast_to([PPART, SPG, w1]),
                )
                e = work.tile([PPART, NCW], f32, tag="e")
                nc.scalar.activation(e[:].bitcast(f32r), an[:], AF.Exp, scale=LAM_SM)
                e_tiles[p] = e

            def stage_b(p):
                """EA/H/EH and the NZ/WZ accumulator matmuls for pack p."""
                first, last = (p == 0), (p == NPACK - 1)
                a_ps = a_tiles.pop(p)
                g_sb = g_all[:, PPART * p:PPART * (p + 1)]
                o_sb = o_all[:, BP - PACK - PACK * p:2 * BP - PACK - PACK * p]
                e = e_tiles.pop(p)

                # EA = E * A  (DVE; reads A from PSUM)
                ea = work.tile([PPART, NCW], f32, tag="ea")
                nc.vector.tensor_mul(ea[:].bitcast(f32r), e[:], a_ps[:])

                # H = Gbd @ E ; NZ += ones_p^T @ EA
                h_ps = psH.tile([PPART, NCW], f32)
                nc.tensor.matmul(
                    h_ps[:], g_sb.bitcast(f32r), e[:].bitcast(f32r),
                    start=True, stop=True,
                )
                nc.tensor.matmul(
                    nz_acc[:], o_sb.bitcast(f32r), ea[:].bitcast(f32r),
                    start=first, stop=last,
                )

                # EH = E * H ; WZ += ones_p^T @ EH
                eh = work.tile([PPART, NCW], f32, tag="eh")
                nc.vector.tensor_mul(eh[:].bitcast(f32r), e[:], h_ps[:])
                nc.tensor.matmul(
                    wz_acc[:], o_sb.bitcast(f32r), eh[:].bitcast(f32r),
                    start=first, stop=last,
                )

            fetch(0)
            load_s()
            for p in range(LOOKAHEAD):
                a_matmul(p)
                if p + 1 < LOOKAHEAD:
                    fetch(p + 1)
            load_go()

            # drained pipeline: iterate p over [0, NPACK+EPI_DELAY) issuing
            # each level for the pack it applies to (guards handle the edges).
            # Order matters per engine stream: stage_b first (ea/eh/H are
            # data-ready at iteration start), sq(p-1) is Pool's stream head
            # (so the sq->reduces->rnrm->an data path never cycles through
    	    # Pool's in-order stream), an/E afterwards.
            for p in range(NPACK + EPI_DELAY):
                if 0 <= p - 3 < NPACK:
                    stage_b(p - 3)
                if p + LOOKAHEAD < NPACK:
                    fetch(p + LOOKAHEAD)
                    a_matmul(p + LOOKAHEAD)
                if p == 0:
                    # cn is only read in phase 2; lowest DMA priority
                    nc.sync.dma_start(cn_sb[:], cn_d[:])
                if p < NPACK:
                    leaky(p)
                if 0 <= p - 1 < NPACK:
                    sq_s2(p - 1)
                    rnrm_stage(p - 1)
                if 0 <= p - 2 < NPACK:
                    an_e(p - 2)

            # ---- phase 2: sim = NZ / max(cn * sqrt(WZ), eps*Z) == NZ/(cn*wn)
            # (Z cancels).  Padded cols have NZ = 0 -> sim = 0 -> ee = 1;
            # the host subtracts the pad counts and takes log()/6.
            # Processed in the two group-column halves so the serial
            # DVE->ACT->DVE chain pipelines across the halves.
            wzm = ph2.tile([BP, NCW], f32)
            lnw = ph2.tile([BP, NCW], f32)
            wn = ph2.tile([BP, NCW], f32)
            den = ph2.tile([BP, NCW], f32)
            den2 = ph2.tile([BP, NCW], f32)
            rden = ph2.tile([BP, NCW], f32)
            simt = ph2.tile([BP, NCW], f32)
            ee = ph2.tile([BP, NCW], f32)
            rowz = ph2.tile([BP, CPC], f32)
            for g, (lo, hi, slo, shi) in enumerate(
                ((0, G0, 0, SPG), (G0, NCW, SPG, CPC))
            ):
                nc.vector.tensor_scalar_max(wzm[:, lo:hi], wz_acc[:, lo:hi], 1e-30)
                nc.scalar.activation(lnw[:, lo:hi], wzm[:, lo:hi], AF.Ln)
                nc.scalar.activation(wn[:, lo:hi], lnw[:, lo:hi], AF.Exp, scale=0.5)
                nc.vector.tensor_mul(den[:, lo:hi], cn_sb[:, lo:hi], wn[:, lo:hi])
                nc.vector.tensor_scalar_max(den2[:, lo:hi], den[:, lo:hi], EPS)
                nc.vector.reciprocal(rden[:, lo:hi], den2[:, lo:hi])
                nc.vector.tensor_mul(simt[:, lo:hi], nz_acc[:, lo:hi], rden[:, lo:hi])
                nc.scalar.activation(ee[:, lo:hi], simt[:, lo:hi], AF.Exp, scale=LAM_LSE)
                # rowZ = sum_w ee (incl. 1.0 per padded word; host corrects)
                nc.vector.tensor_reduce(rowz[:, slo:shi], grp(ee, g), AX.X, ALU.add)

            nc.sync.dma_start(out_d[:], rowz[0:B, :])

    nc.compile()
    restore_tables()
    return nc


def plan_packing(s_l: np.ndarray):
    """Sort captions by length, deal round-robin to (core, slot), and pick the
    two group widths.  Returns (perm[NCORES, CPC] caption ids, w0, w1)."""
    s_l = np.asarray(s_l).astype(np.int64)
    order = np.argsort(-s_l, kind="stable")          # global rank -> caption
    perm = np.empty((NCORES, CPC), np.int64)
    for r, cap in enumerate(order):
        perm[r % NCORES, r // NCORES] = cap
    lens = s_l[order]
    w0 = int(lens[0:SPG * NCORES].max())
    w1 = int(lens[SPG * NCORES:].max())
    # keep NCW >= MIN_NCW for full-rate fp32r matmuls
    short = MIN_NCW - (SPG * w0 + SPG * w1)
    if short > 0:
        w1 += -(-short // SPG)
    w1 = min(w1, W)
    if SPG * (w0 + w1) < MIN_NCW:
        w0 = min(W, w0 + -(-(MIN_NCW - SPG * (w0 + w1)) // SPG))
    return perm, w0, w1


def prepare_inputs(im: np.ndarray, s: np.ndarray, s_l: np.ndarray):
    """Host-side input marshalling: length-balanced ragged caption packing,
    d-major transposes, 3-image/108-partition im packs, block-diagonal Gram
    stationaries, caption norms."""
    im = np.ascontiguousarray(np.asarray(im, np.float32))
    s = np.ascontiguousarray(np.asarray(s, np.float32))
    s_l = np.asarray(s_l).astype(np.int64)

    perm, w0, w1 = plan_packing(s_l)
    G0 = SPG * w0
    NCW = G0 + SPG * w1
    widths = [w0] * SPG + [w1] * SPG
    offs = np.concatenate([[0], np.cumsum(widths)])[:-1]

    # zero out padded words so A columns for padded (c, w) are exactly 0
    wmask = (np.arange(W)[None, :] < s_l[:, None])          # [64, 50]
    s_z = s * wmask[:, :, None].astype(np.float32)
    cn = np.sqrt((s_z * s_z).sum(axis=2))                    # [64, 50]

    # im packs: [22, 128, 8*108]
    imf = im.transpose(2, 0, 1).reshape(D, B * R)            # [1024, 2304]
    imf66 = np.zeros((D, BP * R), np.float32)
    imf66[:, : B * R] = imf
    im_packed = np.ascontiguousarray(
        imf66.reshape(KCH, 128, NPACK, PPART)
        .transpose(2, 1, 0, 3)
        .reshape(NPACK, 128, KCH * PPART)
    )

    # Gram matrices, block-diagonal per pack, concatenated along columns
    # into one [108, 22*108] array (one full-rate DMA)
    G = np.matmul(im, im.transpose(0, 2, 1))                 # [64, 36, 36] f32
    gbd = np.zeros((PPART, NPACK * PPART), np.float32)
    for j in range(PACK):
        for p in range(NPACK):
            b = PACK * p + j
            if b < B:
                gbd[R * j : R * (j + 1),
                    PPART * p + R * j : PPART * p + R * (j + 1)] = G[b]

    # sliding-window ones [108, 132]: 3 block-ones columns at 63..65; pack
    # p's stationary is the [*, 63-3p : 129-3p] slice
    ones_w = np.zeros((PPART, 2 * BP), np.float32)
    for j in range(PACK):
        ones_w[R * j : R * (j + 1), BP - PACK + j] = 1.0

    in_maps = []
    padc = np.empty((NCORES, CPC), np.float32)
    for c in range(NCORES):
        caps = perm[c]                                        # 8 caption ids
        sT = np.zeros((D, NCW), np.float32)
        cn66 = np.zeros((1, NCW), np.float32)
        for j, cap in enumerate(caps):
            L = int(s_l[cap])
            sT[:, offs[j]:offs[j] + L] = s_z[cap, :L].T
            cn66[0, offs[j]:offs[j] + L] = cn[cap, :L]
            padc[c, j] = widths[j] - L
        s_packed = np.ascontiguousarray(
            sT.reshape(KCH, 128, NCW).transpose(1, 0, 2).reshape(128, KCH * NCW)
        )
        in_maps.append(
            {
                "im_packed": im_packed,
                "s_packed": s_packed,
                "gbd": gbd,
                "ones_w": ones_w,
                "cn66": np.ascontiguousarray(
                    np.broadcast_to(cn66, (BP, NCW)), dtype=np.float32
                ),
            }
        )
    return in_maps, padc, perm, (w0, w1)


def margin_loss(scores: np.ndarray) -> np.float32:
    scores = scores.astype(np.float32)
    diag = np.diag(scores).copy()
    cost_s = np.maximum(MARGIN + scores - diag[:, None], 0.0)
    cost_im = np.maximum(MARGIN + scores - diag[None, :], 0.0)
    np.fill_diagonal(cost_s, 0.0)
    np.fill_diagonal(cost_im, 0.0)
    return np.float32(cost_s.max(axis=1).sum() + cost_im.max(axis=0).sum())


def kernel(im: np.ndarray, s: np.ndarray, s_l: np.ndarray) -> np.ndarray:
    from concourse.bass_utils import run_bass_kernel_spmd

    in_maps, padc, perm, key = prepare_inputs(im, s, s_l)
    if key not in _PROGRAM_CACHE:
        _PROGRAM_CACHE[key] = build_program(*key)
    nc = _PROGRAM_CACHE[key]

    res = run_bass_kernel_spmd(nc, in_maps, list(range(NCORES))).results
    scores = np.empty((B, B), np.float32)
    for c in range(NCORES):
        rowz = res[c]["rowz8"]                                # [64, 8]
        scores[:, perm[c]] = np.log(rowz - padc[c][None, :]) / LAM_LSE
    return margin_loss(scores)
